# revision 13
# baseline (speedup 1.0000x reference)
"""CTLSTMCell fused kernel for Trainium2, 8 NeuronCores.

Sharding: tensor-parallel over the D=1024 feature columns. Core c owns
columns [c*128, (c+1)*128) and computes all 7 gate blocks for that slice.
Each core runs the full batch (B=4096); only the shared input x = [emb, h]
is replicated, the weight is split 8 ways and stays resident in SBUF.

On-chip layout is [features, batch] (transposed): the contraction dim K
sits on SBUF partitions for both matmul operands, and the bias lands on
partitions so it fuses into ScalarE activations (func(scale*in+bias)).
Outputs come back [128, 4096] per core and are untransposed on the host.

The tensor engine is the bottleneck and is per-instruction-bound: every
matmul writing a [128, 512] fp32 PSUM tile issues at a fixed ~219 ns
cadence regardless of dtype (PSUM-write rate), so runtime scales with the
number of accumulation rounds per gate tile. A bf16 round contracts K=128;
an fp8e4m3 DoubleRow round contracts K=256 at the same cadence. Rounds per
gate are therefore cut by converting leading K-chunks to fp8, with a
PER-GATE fp8 fraction chosen against the 2e-2 error budget (validated
against the fp32 reference in numpy; the worst output lands at ~1.66e-2):
  - decay gate feeds softplus with SCALE=0.1 and a ~8.5 output scale, so
    it tolerates full fp8 (8 DR rounds, err ~6.6e-3).
  - ig/fg take 5 DR chunks (1280 rows fp8), zg/ibg/fbg 4 chunks.
  - output gate is the most sensitive (sigmoid straight to the output at
    scale ~1): 2 DR chunks only.
Rounds per gate tile: 8+11+11+12+12+12+14 = 80 vs 112 for all-bf16.
To let fp8 and bf16 products accumulate in ONE PSUM bank, all W is
pre-scaled by 2^13 and all x by 2^5 (powers of 2, exact in bf16); the
activation `scale` argument applies the 2^-18 descale for free.

softplus(SCALE*d)/SCALE is a degree-4 polynomial in u = SCALE*d
(|u| <= ~0.35, poly error ~4e-6) staged as three Square activations + two
DVE ops, so ScalarE never swaps activation-table sets.

Initial DMAs are issued in round-consumption order so the first matmul
waits only on the first W/x chunk pair, not a deep issue queue.
"""

import numpy as np

D = 1024
B = 4096
K = 2 * D            # 2048 contraction
NCORES = 8
DLOC = D // NCORES   # 128 columns of D per core
GCOLS = 7 * DLOC     # 896 gate columns per core
KCH8 = 8             # fp8 DoubleRow chunks (K=256 each) cover all of K
KCHB = 12            # bf16 chunks cover rows 512..2048 (og needs them all)
NT = B // 512        # 8 batch tiles of 512
NW = B // 1024       # 4 x-tile windows of 1024
SCALE = 0.1          # softplus beta

# gate -> number of leading DoubleRow (256-row fp8) chunks; the remaining
# rows 256*ndr..2048 run as bf16 chunks (global bf16 chunk i covers rows
# 512+128*i, so gate g uses bf16 chunks 2*ndr-4 .. 11).
GCFG = {0: 5, 1: 5, 2: 2, 3: 4, 4: 4, 5: 4, 6: 8}

SW = 2.0 ** 13       # weight pre-scale (max |W*SW| ~ 181 < 240 fp8e4m3 max)
SX = 2.0 ** 5        # x pre-scale (max |x*SX| ~ 174)
SINV = 1.0 / (SW * SX)

# softplus poly staging constants: with u = SCALE*d and d = SINV*psum + b6,
#   dg = 10*(ln2 + u/2 + u^2/8 - u^4/192)
#      = CPOLY + Square(S1*SINV*psum + [S1*b6 + BQ])
#              - Square(S2 * Square(S1*SINV*psum + S1*b6))
S1 = float(SCALE * np.sqrt(1.25))
S2 = float(np.sqrt(10.0 / 192.0) / 1.25)
CPOLY = float(10.0 * (np.log(2.0) - 0.5))
BQ = float(2.0 * np.sqrt(1.25))

# rounds_of[g]: ordered list of ('8', chunk) then ('b', chunk)
ROUNDS = {
    g: [("8", c) for c in range(ndr)] + [("b", i) for i in range(2 * ndr - 4, KCHB)]
    for g, ndr in GCFG.items()
}

# W chunks are only loaded for the gate columns that actually use them
# (contiguous ranges; gate g owns cols [g*128, (g+1)*128)):
#  - fp8 chunks 5..7 are used only by dg (ndr=8) -> cols 768:896
#  - bf16 chunks 0..3 are used only by og -> cols 256:384
#  - bf16 chunks 4,5 by og/zg/ibg/fbg -> cols 256:768
#  - bf16 chunks 6..11 by all but dg -> cols 0:768
W8CR = {c: (0, GCOLS) if c < 5 else (768, 128) for c in range(KCH8)}
WBCR = {i: ((256, 128) if i < 4 else (256, 512) if i < 6 else (0, 768))
        for i in range(KCHB)}

# chunk-issue order for the initial (window 0) loads: earliest round each
# chunk is first consumed. fp8 chunk c -> round c (dg). bf16 chunk i ->
# round i+2 (og) for i<4, round i (zg/ibg/fbg) for i in {4,5}, round i-1
# (ig/fg) for i>=6.
def _issue_order():
    need = [("8", c, c) for c in range(KCH8)]
    for i in range(KCHB):
        r = i + 2 if i < 4 else (i if i < 6 else i - 1)
        need.append(("b", i, r))
    need.sort(key=lambda t: (t[2], t[0] != "8"))
    return [(k, c) for k, c, _ in need]

ISSUE_ORDER = _issue_order()

_BUILT = {}


def _build():
    import concourse.bacc as bacc
    import concourse.mybir as mybir
    from concourse.tile import TileContext

    bf16 = mybir.dt.bfloat16
    f8 = mybir.dt.float8e4
    f32 = mybir.dt.float32
    AF = mybir.ActivationFunctionType
    DRM = mybir.MatmulPerfMode.DoubleRow

    nc = bacc.Bacc("TRN2")
    x8D = nc.declare_dram_parameter("x8D", [KCH8, 128, 2, B], f8, isOutput=False)
    xT = nc.declare_dram_parameter("xT", [KCHB * 128, B], bf16, isOutput=False)
    W8D = nc.declare_dram_parameter("W8D", [KCH8, 128, 2, GCOLS], f8, isOutput=False)
    Wc = nc.declare_dram_parameter("Wc", [KCHB * 128, GCOLS], bf16, isOutput=False)
    bc = nc.declare_dram_parameter("bc", [DLOC, 8], f32, isOutput=False)
    cellT = nc.declare_dram_parameter("cellT", [DLOC, B], f32, isOutput=False)
    cellbarT = nc.declare_dram_parameter("cellbarT", [DLOC, B], f32, isOutput=False)
    coT = nc.declare_dram_parameter("coT", [DLOC, B], f32, isOutput=True)
    cboT = nc.declare_dram_parameter("cboT", [DLOC, B], f32, isOutput=True)
    dgoT = nc.declare_dram_parameter("dgoT", [DLOC, B], f32, isOutput=True)
    ogoT = nc.declare_dram_parameter("ogoT", [DLOC, B], f32, isOutput=True)

    # dg first: it finishes accumulating earliest (8 rounds), so its 3-ACT
    # chain overlaps the remaining matmuls; og last so only its ACT+store
    # trail the final matmul.
    GORDER = [6, 3, 0, 1, 4, 5, 2]

    with TileContext(nc) as tc:
        with (
            tc.tile_pool(name="wpool", bufs=1) as wp,
            tc.tile_pool(name="xpool", bufs=2) as xp,
            tc.tile_pool(name="gpool", bufs=2) as gp,
            tc.tile_pool(name="tpool", bufs=1) as tp,
            tc.tile_pool(name="opool", bufs=2) as op_,
            tc.tile_pool(name="pspool", bufs=8, space="PSUM") as pp,
        ):
            # x tiles: [*, nb*512] slabs. Window 0 is loaded as two per-n-tile
            # 512-wide slabs so the first n-tile's runway is half as deep;
            # later windows load 1024-wide (2 KB DMA lines) while compute
            # hides them.
            def x8_tile(w0, nb, c):
                ws = slice(w0 * 512, (w0 + nb) * 512)
                xk = xp.tile([128, 2, 1024], f8, tag=f"x8_{c}",
                             name=f"x8_{w0}_{c}")
                nc.sync.dma_start(out=xk[:, :, 0:nb * 512], in_=x8D[c, :, :, ws])
                return xk

            def xb_tile(w0, nb, i):
                ws = slice(w0 * 512, (w0 + nb) * 512)
                xk = xp.tile([128, 1024], bf16, tag=f"x{i}",
                             name=f"x_{w0}_{i}")
                nc.sync.dma_start(out=xk[:, 0:nb * 512],
                                  in_=xT[i * 128:(i + 1) * 128, ws])
                return xk

            def load_x_chunks(w0, nb):
                x8s = [x8_tile(w0, nb, c) for c in range(KCH8)]
                xbs = [xb_tile(w0, nb, i) for i in range(KCHB)]
                return x8s, xbs

            # Window-0 x (first 512 slab) and all W chunks, issued in
            # round-consumption order (W of a chunk just before its x).
            w8s = [None] * KCH8
            wts = [None] * KCHB
            x8s0 = [None] * KCH8
            xbs0 = [None] * KCHB
            for kind, c in ISSUE_ORDER:
                if kind == "8":
                    st, wd = W8CR[c]
                    wk = wp.tile([128, 2, wd], f8, tag=f"w8_{c}", name=f"w8_{c}")
                    nc.sync.dma_start(out=wk[:, :, :],
                                      in_=W8D[c, :, :, st:st + wd])
                    w8s[c] = wk
                    x8s0[c] = x8_tile(0, 1, c)
                else:
                    st, wd = WBCR[c]
                    wk = wp.tile([128, wd], bf16, tag=f"w{c}", name=f"w_{c}")
                    nc.sync.dma_start(
                        out=wk[:, :],
                        in_=Wc[c * 128:(c + 1) * 128, st:st + wd])
                    wts[c] = wk
                    xbs0[c] = xb_tile(0, 1, c)
            xnext = (x8s0, xbs0)

            bt = wp.tile([128, 8], f32)
            nc.sync.dma_start(out=bt[:, :], in_=bc[:, :])

            # x slab schedule in units of 512 batch cols: two single-tile
            # slabs to shorten the window-0 runway, then 1024-wide windows.
            SLABS = [(0, 1), (1, 1), (2, 2), (4, 2), (6, 2)]
            slab_idx = 0
            x8s, xbs = xnext
            xnext = load_x_chunks(*SLABS[1])

            for n in range(NT):
                ns = slice(n * 512, (n + 1) * 512)
                if slab_idx + 1 < len(SLABS) and n == SLABS[slab_idx + 1][0]:
                    slab_idx += 1
                    x8s, xbs = xnext
                    if slab_idx + 1 < len(SLABS):
                        xnext = load_x_chunks(*SLABS[slab_idx + 1])
                local = n - SLABS[slab_idx][0]
                hs = slice(local * 512, (local + 1) * 512)

                ct = gp.tile([128, 512], f32, tag="ct")
                nc.sync.dma_start(out=ct[:, :], in_=cellT[:, ns])
                cbt = gp.tile([128, 512], f32, tag="cbt")
                nc.sync.dma_start(out=cbt[:, :], in_=cellbarT[:, ns])

                # round-outer, gate-inner: PSUM banks accumulate in lockstep,
                # paced by the chunk DMAs. The last n-tile runs gate-outer so
                # only og's ACT+store trail the final matmul.
                pts = {
                    g: pp.tile([128, 512], f32, tag="pt", name=f"pt_{n}_{g}")
                    for g in GORDER
                }
                if n < NT - 1:
                    # og's rounds are deferred ~6 slots so every other gate
                    # stops earlier: their ACT drains start sooner, freeing
                    # PSUM banks before the next tile's round-0 needs them.
                    items = []
                    for pos, g in enumerate(GORDER):
                        for r in range(len(ROUNDS[g])):
                            items.append((r + (6 if g == 2 else 0), pos, r, g))
                    items.sort()
                    loop = [(r, g) for _, _, r, g in items]
                else:
                    loop = [(r, g) for g in GORDER for r in range(len(ROUNDS[g]))]
                for r, g in loop:
                    kind, c = ROUNDS[g][r]
                    last = r == len(ROUNDS[g]) - 1
                    if kind == "8":
                        st, _ = W8CR[c]
                        lo = g * 128 - st
                        nc.tensor.matmul(
                            pts[g][:, :],
                            w8s[c][:, :, lo:lo + 128],
                            x8s[c][:, :, hs],
                            start=(r == 0),
                            stop=last,
                            perf_mode=DRM,
                        )
                    else:
                        st, _ = WBCR[c]
                        lo = g * 128 - st
                        nc.tensor.matmul(
                            pts[g][:, :],
                            wts[c][:, lo:lo + 128],
                            xbs[c][:, hs],
                            start=(r == 0),
                            stop=last,
                        )

                # decay gate: polynomial softplus (see constants above).
                # bc[:, 6] = S1*b6 + BQ, bc[:, 7] = S1*b6 (host-prepped).
                qg = tp.tile([128, 512], f32, tag="qg")
                nc.scalar.activation(
                    qg[:, :], pts[6][:, :], AF.Square, bias=bt[:, 6:7],
                    scale=S1 * SINV,
                )
                rg = gp.tile([128, 512], f32, tag="rg")
                nc.scalar.activation(
                    rg[:, :], pts[6][:, :], AF.Square, bias=bt[:, 7:8],
                    scale=S1 * SINV,
                )
                u4s = tp.tile([128, 512], f32, tag="u4s")
                nc.scalar.activation(u4s[:, :], rg[:, :], AF.Square, scale=S2)
                tsum = gp.tile([128, 512], f32, tag="tsum")
                nc.vector.tensor_sub(tsum[:, :], qg[:, :], u4s[:, :])
                dgt = op_.tile([128, 512], f32, tag="dgt")
                nc.vector.tensor_scalar_add(dgt[:, :], tsum[:, :], CPOLY)
                nc.sync.dma_start(out=dgoT[:, ns], in_=dgt[:, :])

                cin = gp.tile([128, 512], f32, tag="cin")
                nc.scalar.activation(cin[:, :], pts[3][:, :], AF.Tanh,
                                     bias=bt[:, 3:4], scale=SINV)
                s_ig = gp.tile([128, 512], f32, tag="s_ig")
                nc.scalar.activation(s_ig[:, :], pts[0][:, :], AF.Sigmoid,
                                     bias=bt[:, 0:1], scale=SINV)
                s_fg = gp.tile([128, 512], f32, tag="s_fg")
                nc.scalar.activation(s_fg[:, :], pts[1][:, :], AF.Sigmoid,
                                     bias=bt[:, 1:2], scale=SINV)

                t1 = tp.tile([128, 512], f32, tag="t1")
                nc.vector.tensor_mul(t1[:, :], s_fg[:, :], ct[:, :])
                t2 = tp.tile([128, 512], f32, tag="t2")
                nc.vector.tensor_mul(t2[:, :], s_ig[:, :], cin[:, :])
                cot = op_.tile([128, 512], f32, tag="cot")
                nc.vector.tensor_add(cot[:, :], t1[:, :], t2[:, :])
                nc.sync.dma_start(out=coT[:, ns], in_=cot[:, :])

                s_ibg = gp.tile([128, 512], f32, tag="s_ibg")
                nc.scalar.activation(s_ibg[:, :], pts[4][:, :], AF.Sigmoid,
                                     bias=bt[:, 4:5], scale=SINV)
                s_fbg = gp.tile([128, 512], f32, tag="s_fbg")
                nc.scalar.activation(s_fbg[:, :], pts[5][:, :], AF.Sigmoid,
                                     bias=bt[:, 5:6], scale=SINV)

                t3 = tp.tile([128, 512], f32, tag="t3")
                nc.vector.tensor_mul(t3[:, :], s_fbg[:, :], cbt[:, :])
                t4 = tp.tile([128, 512], f32, tag="t4")
                nc.vector.tensor_mul(t4[:, :], s_ibg[:, :], cin[:, :])
                cbot = op_.tile([128, 512], f32, tag="cbot")
                nc.vector.tensor_add(cbot[:, :], t3[:, :], t4[:, :])
                nc.sync.dma_start(out=cboT[:, ns], in_=cbot[:, :])

                ogt = op_.tile([128, 512], f32, tag="ogt")
                nc.scalar.activation(ogt[:, :], pts[2][:, :], AF.Sigmoid,
                                     bias=bt[:, 2:3], scale=SINV)
                nc.sync.dma_start(out=ogoT[:, ns], in_=ogt[:, :])

    nc.compile()
    return nc


def get_nc():
    if "nc" not in _BUILT:
        _BUILT["nc"] = _build()
    return _BUILT["nc"]


def make_in_maps(event_type_emb_i, hidden_t__i_minus_1, cell_t__i_minus_1,
                 cell_bar_i_minus_1, W, b):
    import ml_dtypes

    emb = np.asarray(event_type_emb_i, dtype=np.float32)
    h = np.asarray(hidden_t__i_minus_1, dtype=np.float32)
    cell = np.asarray(cell_t__i_minus_1, dtype=np.float32)
    cellbar = np.asarray(cell_bar_i_minus_1, dtype=np.float32)
    W = np.asarray(W, dtype=np.float32)
    b = np.asarray(b, dtype=np.float32)

    xTf = np.concatenate([emb, h], axis=1).T * SX  # [2048, 4096], pre-scaled
    # fp8: all rows, packed [c, p, i, b] with k = 256c + 128i + p
    x8 = np.ascontiguousarray(
        xTf.reshape(KCH8, 2, 128, B).transpose(0, 2, 1, 3)
        .astype(ml_dtypes.float8_e4m3)
    )
    # bf16: rows 512..2048
    xT = np.ascontiguousarray(xTf[512:].astype(ml_dtypes.bfloat16))
    cellT = np.ascontiguousarray(cell.T)        # [1024, 4096]
    cellbarT = np.ascontiguousarray(cellbar.T)  # [1024, 4096]

    in_maps = []
    for c in range(NCORES):
        cols = np.concatenate(
            [np.arange(g * D + c * DLOC, g * D + (c + 1) * DLOC) for g in range(7)]
        )
        Wf = W[:, cols] * SW  # [2048, 896], pre-scaled
        W8 = np.ascontiguousarray(
            Wf.reshape(KCH8, 2, 128, GCOLS).transpose(0, 2, 1, 3)
            .astype(ml_dtypes.float8_e4m3)
        )
        Wcb = np.ascontiguousarray(Wf[512:].astype(ml_dtypes.bfloat16))
        b7 = b[cols].reshape(7, DLOC).T  # [128, 7]
        bc = np.empty((DLOC, 8), dtype=np.float32)
        bc[:, :6] = b7[:, :6]
        bc[:, 6] = S1 * b7[:, 6] + BQ
        bc[:, 7] = S1 * b7[:, 6]
        in_maps.append({
            "x8D": x8,
            "xT": xT,
            "W8D": W8,
            "Wc": Wcb,
            "bc": bc,
            "cellT": np.ascontiguousarray(cellT[c * DLOC:(c + 1) * DLOC, :]),
            "cellbarT": np.ascontiguousarray(cellbarT[c * DLOC:(c + 1) * DLOC, :]),
        })
    return in_maps


def assemble(results):
    outs = []
    for name in ("coT", "cboT", "dgoT", "ogoT"):
        full = np.empty((B, D), dtype=np.float32)
        for c, r in enumerate(results):
            full[:, c * DLOC:(c + 1) * DLOC] = r[name].T
        outs.append(full)
    return tuple(outs)


def kernel(**inputs):
    from concourse.bass_utils import run_bass_kernel_spmd

    nc = get_nc()
    in_maps = make_in_maps(**inputs)
    res = run_bass_kernel_spmd(nc, in_maps, list(range(NCORES)))
    return assemble(res.results)


# revision 20
# speedup vs baseline: 1.1870x; 1.1870x over previous
"""CTLSTMCell fused kernel for Trainium2, 8 NeuronCores.

Sharding: tensor-parallel over the D=1024 feature columns. Core c owns
columns [c*128, (c+1)*128) and computes all 7 gate blocks for that slice.
Each core runs the full batch (B=4096); only the shared input x = [emb, h]
is replicated, the weight is split 8 ways and stays resident in SBUF.

On-chip layout is [features, batch] (transposed): the contraction dim K
sits on SBUF partitions for both matmul operands, and the bias lands on
partitions so it fuses into ScalarE activations (func(scale*in+bias)).
Outputs come back [128, 4096] per core and are untransposed on the host.

The tensor engine is the bottleneck and is per-instruction-bound: every
matmul writing a [128, 512] fp32 PSUM tile issues at a fixed ~219 ns
cadence regardless of dtype (PSUM-write rate), so runtime scales with the
number of accumulation rounds per gate tile. A bf16 round contracts K=128;
an fp8e4m3 DoubleRow round contracts K=256 at the same cadence. Rounds per
gate are therefore cut by converting leading K-chunks to fp8, with a
PER-GATE fp8 fraction chosen against the 2e-2 error budget (validated
against the fp32 reference in numpy; the worst output lands at ~1.66e-2):
  - decay gate feeds softplus with SCALE=0.1 and a ~8.5 output scale, so
    it tolerates full fp8 (8 DR rounds, err ~6.6e-3).
  - ig/fg take 5 DR chunks (1280 rows fp8), zg/ibg/fbg 4 chunks.
  - output gate is the most sensitive (sigmoid straight to the output at
    scale ~1): 2 DR chunks only.
Rounds per gate tile: 8+11+11+12+12+12+14 = 80 vs 112 for all-bf16.
To let fp8 and bf16 products accumulate in ONE PSUM bank, all W is
pre-scaled by 2^13 and all x by 2^5 (powers of 2, exact in bf16); the
activation `scale` argument applies the 2^-18 descale for free.

softplus(SCALE*d)/SCALE is a degree-4 polynomial in u = SCALE*d
(|u| <= ~0.35, poly error ~4e-6) staged as three Square activations + two
DVE ops, so ScalarE never swaps activation-table sets.

Initial DMAs are issued in round-consumption order so the first matmul
waits only on the first W/x chunk pair, not a deep issue queue.
"""

import numpy as np

D = 1024
B = 4096
K = 2 * D            # 2048 contraction
NCORES = 8
DLOC = D // NCORES   # 128 columns of D per core
GCOLS = 7 * DLOC     # 896 gate columns per core
KCH8 = 8             # fp8 DoubleRow chunks (K=256 each) cover all of K
KCHB = 12            # bf16 chunks cover rows 512..2048 (og needs them all)
NT = B // 512        # 8 batch tiles of 512
NW = B // 1024       # 4 x-tile windows of 1024
SCALE = 0.1          # softplus beta

# gate -> number of leading DoubleRow (256-row fp8) chunks; the remaining
# rows 256*ndr..2048 run as bf16 chunks (global bf16 chunk i covers rows
# 512+128*i, so gate g uses bf16 chunks 2*ndr-4 .. 11).
GCFG = {0: 5, 1: 5, 2: 2, 3: 4, 4: 4, 5: 4, 6: 8}

SW = 2.0 ** 13       # weight pre-scale (max |W*SW| ~ 181 < 240 fp8e4m3 max)
SX = 2.0 ** 5        # x pre-scale (max |x*SX| ~ 174)
SINV = 1.0 / (SW * SX)

# softplus poly staging constants: with u = SCALE*d and d = SINV*psum + b6,
#   dg = 10*(ln2 + u/2 + u^2/8 - u^4/192)
#      = CPOLY + Square(S1*SINV*psum + [S1*b6 + BQ])
#              - Square(S2 * Square(S1*SINV*psum + S1*b6))
S1 = float(SCALE * np.sqrt(1.25))
S2 = float(np.sqrt(10.0 / 192.0) / 1.25)
CPOLY = float(10.0 * (np.log(2.0) - 0.5))
BQ = float(2.0 * np.sqrt(1.25))

# rounds_of[g]: ordered list of ('8', chunk) then ('b', chunk)
ROUNDS = {
    g: [("8", c) for c in range(ndr)] + [("b", i) for i in range(2 * ndr - 4, KCHB)]
    for g, ndr in GCFG.items()
}

# bf16 W chunks are only loaded for the gate columns that actually use them
# (contiguous ranges; gate g owns cols [g*128, (g+1)*128)):
#  - bf16 chunks 0..3 are used only by og -> cols 256:384
#  - bf16 chunks 4,5 by og/zg/ibg/fbg -> cols 256:768
#  - bf16 chunks 6..11 by all but dg -> cols 0:768
# fp8 chunks stay full-width: a trimmed [128, 2, 128] slice DMAs as 128-byte
# lines, which costs more in descriptor overhead than the bytes it saves.
W8CR = {c: (0, GCOLS) for c in range(KCH8)}
WBCR = {i: ((256, 128) if i < 4 else (256, 512) if i < 6 else (0, 768))
        for i in range(KCHB)}

# chunk-issue order for the initial (window 0) loads: earliest round each
# chunk is first consumed. fp8 chunk c -> round c (dg). bf16 chunk i ->
# round i+2 (og) for i<4, round i (zg/ibg/fbg) for i in {4,5}, round i-1
# (ig/fg) for i>=6.
def _issue_order():
    need = [("8", c, c) for c in range(KCH8)]
    for i in range(KCHB):
        r = i + 2 if i < 4 else (i if i < 6 else i - 1)
        need.append(("b", i, r))
    need.sort(key=lambda t: (t[2], t[0] != "8"))
    return [(k, c) for k, c, _ in need]

ISSUE_ORDER = _issue_order()

_BUILT = {}


def _build():
    import concourse.bacc as bacc
    import concourse.mybir as mybir
    from concourse.tile import TileContext

    bf16 = mybir.dt.bfloat16
    f8 = mybir.dt.float8e4
    f32 = mybir.dt.float32
    AF = mybir.ActivationFunctionType
    DRM = mybir.MatmulPerfMode.DoubleRow

    nc = bacc.Bacc("TRN2")
    # x8D is packed per 1024-col window so each window DMA reads one
    # contiguous 2 KB line per partition: x8D[c, p, w, i, j] =
    # x_fp8[k = 256c + 128i + p, b = 1024w + j].
    x8D = nc.declare_dram_parameter("x8D", [KCH8, 128, NW, 2, 1024], f8,
                                    isOutput=False)
    xT = nc.declare_dram_parameter("xT", [KCHB * 128, B], bf16, isOutput=False)
    W8D = nc.declare_dram_parameter("W8D", [KCH8, 128, 2, GCOLS], f8, isOutput=False)
    Wc = nc.declare_dram_parameter("Wc", [KCHB * 128, GCOLS], bf16, isOutput=False)
    bc = nc.declare_dram_parameter("bc", [DLOC, 8], f32, isOutput=False)
    cellT = nc.declare_dram_parameter("cellT", [DLOC, B], f32, isOutput=False)
    cellbarT = nc.declare_dram_parameter("cellbarT", [DLOC, B], f32, isOutput=False)
    coT = nc.declare_dram_parameter("coT", [DLOC, B], f32, isOutput=True)
    cboT = nc.declare_dram_parameter("cboT", [DLOC, B], f32, isOutput=True)
    dgoT = nc.declare_dram_parameter("dgoT", [DLOC, B], f32, isOutput=True)
    ogoT = nc.declare_dram_parameter("ogoT", [DLOC, B], f32, isOutput=True)

    # dg first: it finishes accumulating earliest (8 rounds), so its 3-ACT
    # chain overlaps the remaining matmuls; og last so only its ACT+store
    # trail the final matmul.
    GORDER = [6, 3, 0, 1, 4, 5, 2]

    with TileContext(nc) as tc:
        with (
            tc.tile_pool(name="wpool", bufs=1) as wp,
            tc.tile_pool(name="xpool", bufs=2) as xp,
            tc.tile_pool(name="gpool", bufs=2) as gp,
            tc.tile_pool(name="tpool", bufs=1) as tp,
            tc.tile_pool(name="opool", bufs=2) as op_,
            tc.tile_pool(name="pspool", bufs=8, space="PSUM") as pp,
        ):
            # x tiles: one [*, 1024] window per chunk (2 KB DMA lines).
            def x8_tile(w, c):
                xk = xp.tile([128, 2, 1024], f8, tag=f"x8_{c}",
                             name=f"x8_{w}_{c}")
                nc.sync.dma_start(out=xk[:, :, :], in_=x8D[c, :, w, :, :])
                return xk

            def xb_tile(w, i):
                ws = slice(w * 1024, (w + 1) * 1024)
                xk = xp.tile([128, 1024], bf16, tag=f"x{i}",
                             name=f"x_{w}_{i}")
                nc.sync.dma_start(out=xk[:, :],
                                  in_=xT[i * 128:(i + 1) * 128, ws])
                return xk

            def load_x_chunks(w):
                x8s = [x8_tile(w, c) for c in range(KCH8)]
                xbs = [xb_tile(w, i) for i in range(KCHB)]
                return x8s, xbs

            # Window-0 x and all W chunks, issued in round-consumption order
            # (W of a chunk just before its x).
            w8s = [None] * KCH8
            wts = [None] * KCHB
            x8s0 = [None] * KCH8
            xbs0 = [None] * KCHB
            for kind, c in ISSUE_ORDER:
                if kind == "8":
                    st, wd = W8CR[c]
                    wk = wp.tile([128, 2, wd], f8, tag=f"w8_{c}", name=f"w8_{c}")
                    nc.sync.dma_start(out=wk[:, :, :],
                                      in_=W8D[c, :, :, st:st + wd])
                    w8s[c] = wk
                    x8s0[c] = x8_tile(0, c)
                else:
                    st, wd = WBCR[c]
                    wk = wp.tile([128, wd], bf16, tag=f"w{c}", name=f"w_{c}")
                    nc.sync.dma_start(
                        out=wk[:, :],
                        in_=Wc[c * 128:(c + 1) * 128, st:st + wd])
                    wts[c] = wk
                    xbs0[c] = xb_tile(0, c)
            xnext = (x8s0, xbs0)

            bt = wp.tile([128, 8], f32)
            nc.sync.dma_start(out=bt[:, :], in_=bc[:, :])

            for n in range(NT):
                w, half = divmod(n, 2)
                ns = slice(n * 512, (n + 1) * 512)
                hs = slice(half * 512, (half + 1) * 512)
                if half == 0:
                    x8s, xbs = xnext
                    if w + 1 < NW:
                        xnext = load_x_chunks(w + 1)

                ct = gp.tile([128, 512], f32, tag="ct")
                nc.sync.dma_start(out=ct[:, :], in_=cellT[:, ns])
                cbt = gp.tile([128, 512], f32, tag="cbt")
                nc.sync.dma_start(out=cbt[:, :], in_=cellbarT[:, ns])

                # round-outer, gate-inner: PSUM banks accumulate in lockstep,
                # paced by the chunk DMAs. The last n-tile runs gate-outer so
                # only og's ACT+store trail the final matmul.
                pts = {
                    g: pp.tile([128, 512], f32, tag="pt", name=f"pt_{n}_{g}")
                    for g in GORDER
                }
                if n < NT - 1:
                    # og's rounds are deferred ~6 slots so every other gate
                    # stops earlier: their ACT drains start sooner, freeing
                    # PSUM banks before the next tile's round-0 needs them.
                    items = []
                    for pos, g in enumerate(GORDER):
                        for r in range(len(ROUNDS[g])):
                            items.append((r + (6 if g == 2 else 0), pos, r, g))
                    items.sort()
                    loop = [(r, g) for _, _, r, g in items]
                else:
                    loop = [(r, g) for g in GORDER for r in range(len(ROUNDS[g]))]
                for r, g in loop:
                    kind, c = ROUNDS[g][r]
                    last = r == len(ROUNDS[g]) - 1
                    if kind == "8":
                        st, _ = W8CR[c]
                        lo = g * 128 - st
                        nc.tensor.matmul(
                            pts[g][:, :],
                            w8s[c][:, :, lo:lo + 128],
                            x8s[c][:, :, hs],
                            start=(r == 0),
                            stop=last,
                            perf_mode=DRM,
                        )
                    else:
                        st, _ = WBCR[c]
                        lo = g * 128 - st
                        nc.tensor.matmul(
                            pts[g][:, :],
                            wts[c][:, lo:lo + 128],
                            xbs[c][:, hs],
                            start=(r == 0),
                            stop=last,
                        )

                # decay gate: polynomial softplus (see constants above).
                # bc[:, 6] = S1*b6 + BQ, bc[:, 7] = S1*b6 (host-prepped).
                qg = tp.tile([128, 512], f32, tag="qg")
                nc.scalar.activation(
                    qg[:, :], pts[6][:, :], AF.Square, bias=bt[:, 6:7],
                    scale=S1 * SINV,
                )
                rg = gp.tile([128, 512], f32, tag="rg")
                nc.scalar.activation(
                    rg[:, :], pts[6][:, :], AF.Square, bias=bt[:, 7:8],
                    scale=S1 * SINV,
                )
                u4s = tp.tile([128, 512], f32, tag="u4s")
                nc.scalar.activation(u4s[:, :], rg[:, :], AF.Square, scale=S2)
                tsum = gp.tile([128, 512], f32, tag="tsum")
                nc.vector.tensor_sub(tsum[:, :], qg[:, :], u4s[:, :])
                dgt = op_.tile([128, 512], f32, tag="dgt")
                nc.vector.tensor_scalar_add(dgt[:, :], tsum[:, :], CPOLY)
                nc.sync.dma_start(out=dgoT[:, ns], in_=dgt[:, :])

                cin = gp.tile([128, 512], f32, tag="cin")
                nc.scalar.activation(cin[:, :], pts[3][:, :], AF.Tanh,
                                     bias=bt[:, 3:4], scale=SINV)
                s_ig = gp.tile([128, 512], f32, tag="s_ig")
                nc.scalar.activation(s_ig[:, :], pts[0][:, :], AF.Sigmoid,
                                     bias=bt[:, 0:1], scale=SINV)
                s_fg = gp.tile([128, 512], f32, tag="s_fg")
                nc.scalar.activation(s_fg[:, :], pts[1][:, :], AF.Sigmoid,
                                     bias=bt[:, 1:2], scale=SINV)

                t1 = tp.tile([128, 512], f32, tag="t1")
                nc.vector.tensor_mul(t1[:, :], s_fg[:, :], ct[:, :])
                t2 = tp.tile([128, 512], f32, tag="t2")
                nc.vector.tensor_mul(t2[:, :], s_ig[:, :], cin[:, :])
                cot = op_.tile([128, 512], f32, tag="cot")
                nc.vector.tensor_add(cot[:, :], t1[:, :], t2[:, :])
                nc.sync.dma_start(out=coT[:, ns], in_=cot[:, :])

                s_ibg = gp.tile([128, 512], f32, tag="s_ibg")
                nc.scalar.activation(s_ibg[:, :], pts[4][:, :], AF.Sigmoid,
                                     bias=bt[:, 4:5], scale=SINV)
                s_fbg = gp.tile([128, 512], f32, tag="s_fbg")
                nc.scalar.activation(s_fbg[:, :], pts[5][:, :], AF.Sigmoid,
                                     bias=bt[:, 5:6], scale=SINV)

                t3 = tp.tile([128, 512], f32, tag="t3")
                nc.vector.tensor_mul(t3[:, :], s_fbg[:, :], cbt[:, :])
                t4 = tp.tile([128, 512], f32, tag="t4")
                nc.vector.tensor_mul(t4[:, :], s_ibg[:, :], cin[:, :])
                cbot = op_.tile([128, 512], f32, tag="cbot")
                nc.vector.tensor_add(cbot[:, :], t3[:, :], t4[:, :])
                nc.sync.dma_start(out=cboT[:, ns], in_=cbot[:, :])

                ogt = op_.tile([128, 512], f32, tag="ogt")
                nc.scalar.activation(ogt[:, :], pts[2][:, :], AF.Sigmoid,
                                     bias=bt[:, 2:3], scale=SINV)
                nc.sync.dma_start(out=ogoT[:, ns], in_=ogt[:, :])

    nc.compile()
    return nc


def get_nc():
    if "nc" not in _BUILT:
        _BUILT["nc"] = _build()
    return _BUILT["nc"]


def make_in_maps(event_type_emb_i, hidden_t__i_minus_1, cell_t__i_minus_1,
                 cell_bar_i_minus_1, W, b):
    import ml_dtypes

    emb = np.asarray(event_type_emb_i, dtype=np.float32)
    h = np.asarray(hidden_t__i_minus_1, dtype=np.float32)
    cell = np.asarray(cell_t__i_minus_1, dtype=np.float32)
    cellbar = np.asarray(cell_bar_i_minus_1, dtype=np.float32)
    W = np.asarray(W, dtype=np.float32)
    b = np.asarray(b, dtype=np.float32)

    xTf = np.concatenate([emb, h], axis=1).T * SX  # [2048, 4096], pre-scaled
    # fp8: all rows, packed [c, p, w, i, j] with k = 256c + 128i + p,
    # b = 1024w + j (one contiguous 2 KB line per partition per window)
    x8 = np.ascontiguousarray(
        xTf.reshape(KCH8, 2, 128, NW, 1024).transpose(0, 2, 3, 1, 4)
        .astype(ml_dtypes.float8_e4m3)
    )
    # bf16: rows 512..2048
    xT = np.ascontiguousarray(xTf[512:].astype(ml_dtypes.bfloat16))
    cellT = np.ascontiguousarray(cell.T)        # [1024, 4096]
    cellbarT = np.ascontiguousarray(cellbar.T)  # [1024, 4096]

    in_maps = []
    for c in range(NCORES):
        cols = np.concatenate(
            [np.arange(g * D + c * DLOC, g * D + (c + 1) * DLOC) for g in range(7)]
        )
        Wf = W[:, cols] * SW  # [2048, 896], pre-scaled
        W8 = np.ascontiguousarray(
            Wf.reshape(KCH8, 2, 128, GCOLS).transpose(0, 2, 1, 3)
            .astype(ml_dtypes.float8_e4m3)
        )
        Wcb = np.ascontiguousarray(Wf[512:].astype(ml_dtypes.bfloat16))
        b7 = b[cols].reshape(7, DLOC).T  # [128, 7]
        bc = np.empty((DLOC, 8), dtype=np.float32)
        bc[:, :6] = b7[:, :6]
        bc[:, 6] = S1 * b7[:, 6] + BQ
        bc[:, 7] = S1 * b7[:, 6]
        in_maps.append({
            "x8D": x8,
            "xT": xT,
            "W8D": W8,
            "Wc": Wcb,
            "bc": bc,
            "cellT": np.ascontiguousarray(cellT[c * DLOC:(c + 1) * DLOC, :]),
            "cellbarT": np.ascontiguousarray(cellbarT[c * DLOC:(c + 1) * DLOC, :]),
        })
    return in_maps


def assemble(results):
    outs = []
    for name in ("coT", "cboT", "dgoT", "ogoT"):
        full = np.empty((B, D), dtype=np.float32)
        for c, r in enumerate(results):
            full[:, c * DLOC:(c + 1) * DLOC] = r[name].T
        outs.append(full)
    return tuple(outs)


def kernel(**inputs):
    from concourse.bass_utils import run_bass_kernel_spmd

    nc = get_nc()
    in_maps = make_in_maps(**inputs)
    res = run_bass_kernel_spmd(nc, in_maps, list(range(NCORES)))
    return assemble(res.results)
